# revision 14
# baseline (speedup 1.0000x reference)
"""Multi-head attention (B=2, S=2048, D=1024, H=16) on 8 Trainium2 NeuronCores.

Sharding: data-parallel over batch (2) x tensor-parallel over heads (4 head
groups of 4 heads) -> 8 shards, one per core.  Attention is fully independent
per (batch, head); only the output projection needs a cross-head reduction,
which is done host-side (4 partial sums of [S, D] per batch).

Per-core device kernel (all fp32):
  phase 1: QhT = (Wq*0.125)^T @ q^T (+bq), KhT = Wk^T @ k^T (+bk)  [65,4,S]
           (row 64 of QhT = 1.0, row 64 of KhT = mask * -30000 -> the
           augmented K=65 matmul adds the additive mask for free)
           Vh  = v @ Wv (+bv)  stored [128, kt, h, 64]
  phase 2: per head: logits in both orientations via PE matmuls (K=65),
           exp on ACT (accum_out gives softmax row-sums for free),
           attn = exp * recip  -> DMA out;  ctxT = Vh^T @ expT, scaled by
           recip broadcast via a tiny PE transpose + ones-matmul.
  phase 3: out_part = ctx @ Wo_rows  (K=64 per head chunk) -> DMA out.
"""

import os
from contextlib import ExitStack

import numpy as np

import concourse.bass as bass
import concourse.mybir as mybir
import concourse.tile as tile
from concourse import bacc, bass_utils
from concourse.masks import make_identity

F32 = mybir.dt.float32
Exp = mybir.ActivationFunctionType.Exp

B, S, DIN, DM, H = 2, 2048, 1024, 1024, 16
DEP = 64
HPC = 4          # heads per core
NCORES = 8
PEN = -30000.0   # additive mask penalty; exp(x + PEN) == 0.0 exactly in f32
IC = DIN // 128  # input-dim chunks


def _emit(ctx: ExitStack, tc: "tile.TileContext", io: dict, s: int, phases: int = 7):
    nc = tc.nc
    KT = s // 128       # k tiles (also s tiles)
    QT = s // 128       # q tiles
    C5 = s // 512       # 512-wide chunks
    HB = s // 256       # projection column blocks

    const = ctx.enter_context(tc.tile_pool(name="const", bufs=1))
    persist = ctx.enter_context(tc.tile_pool(name="persist", bufs=1))
    stage = ctx.enter_context(tc.tile_pool(name="stage", bufs=4))
    work = ctx.enter_context(tc.tile_pool(name="work", bufs=3))
    ps_big = ctx.enter_context(tc.tile_pool(name="ps_big", bufs=2, space="PSUM"))
    ps_ctx = ctx.enter_context(tc.tile_pool(name="ps_ctx", bufs=2, space="PSUM"))
    ps_med = ctx.enter_context(tc.tile_pool(name="ps_med", bufs=2, space="PSUM"))

    # ---- constants -------------------------------------------------------
    wq_sb = const.tile([128, IC, HPC * DEP], F32)
    wk_sb = const.tile([128, IC, HPC * DEP], F32)
    wv_sb = const.tile([128, IC, HPC * DEP], F32)
    for t, name in ((wq_sb, "wq"), (wk_sb, "wk"), (wv_sb, "wv")):
        nc.sync.dma_start(out=t, in_=io[name].rearrange("(ic p) c -> p ic c", p=128))
    bq_sb = const.tile([1, HPC * DEP], F32)
    bk_sb = const.tile([1, HPC * DEP], F32)
    bv_sb = const.tile([1, HPC * DEP], F32)
    for t, name in ((bq_sb, "bq"), (bk_sb, "bk"), (bv_sb, "bv")):
        nc.sync.dma_start(out=t, in_=io[name])
    ones_row = const.tile([1, 512], F32)
    nc.vector.memset(ones_row, 1.0)
    id_sb = const.tile([128, 128], F32)
    make_identity(nc, id_sb)

    # ---- persistent state ------------------------------------------------
    QhT = persist.tile([65, HPC, s], F32)     # row 64 = 1.0
    KhT = persist.tile([65, HPC, s], F32)     # row 64 = penalty
    Vh = persist.tile([128, KT, HPC, DEP], F32)
    ctxT = persist.tile([128, 2, s], F32)     # [sub*64+d, pair, q]
    rcol = persist.tile([128, HPC, QT], F32)  # recip of softmax row sums

    nc.vector.memset(QhT[64:65, :, :], 1.0)
    for h in range(HPC):
        nc.sync.dma_start(out=KhT[64:65, h, :], in_=io["pen"])

    # ---- phase 1: projections -------------------------------------------
    for hb in range(HB):
        qt_t = stage.tile([128, IC, 256], F32, tag="stage")
        kt_t = stage.tile([128, IC, 256], F32, tag="stage")
        vt_t = stage.tile([128, IC, 256], F32, tag="stage")
        for t, name in ((qt_t, "qT"), (kt_t, "kT"), (vt_t, "vT")):
            src = io[name].rearrange("(ic p) (hb c) -> p ic hb c", p=128, c=256)
            nc.sync.dma_start(out=t, in_=src[:, :, hb, :])

        for src_t, w_sb, b_sb, dst in ((qt_t, wq_sb, bq_sb, QhT), (kt_t, wk_sb, bk_sb, KhT)):
            for h in range(HPC):
                ps_p = ps_med.tile([64, 256], F32, tag="med")
                for ic in range(IC):
                    nc.tensor.matmul(
                        ps_p, lhsT=w_sb[:, ic, h * DEP:(h + 1) * DEP],
                        rhs=src_t[:, ic, :], start=(ic == 0), stop=False)
                nc.tensor.matmul(
                    ps_p, lhsT=b_sb[0:1, h * DEP:(h + 1) * DEP],
                    rhs=ones_row[0:1, 0:256], start=False, stop=True)
                nc.vector.tensor_copy(out=dst[0:64, h, hb * 256:(hb + 1) * 256], in_=ps_p)

        for st in range(2):
            ps_v = ps_med.tile([128, 256], F32, tag="med")
            for ic in range(IC):
                nc.tensor.matmul(
                    ps_v, lhsT=vt_t[:, ic, st * 128:(st + 1) * 128],
                    rhs=wv_sb[:, ic, :], start=(ic == 0), stop=False)
            nc.tensor.matmul(
                ps_v, lhsT=ones_row[0:1, 0:128], rhs=bv_sb[0:1, :],
                start=False, stop=True)
            kt = hb * 2 + st
            for h in range(HPC):
                nc.vector.tensor_copy(out=Vh[:, kt, h, :], in_=ps_v[:, h * DEP:(h + 1) * DEP])

    # ---- phase 2: attention ---------------------------------------------
    if not (phases & 2):
        return
    nat_on = not bool(phases & 8)
    t_on = not bool(phases & 16)
    if not nat_on:
        nc.vector.memset(rcol, 1.0)
    for pp in range(HPC // 2):
        heads = (2 * pp, 2 * pp + 1)
        # natural orientation: exp + row sums + attn output
        for h in heads if nat_on else ():
            for qt in range(QT):
                exp_slab = work.tile([128, s], F32, tag="exp", bufs=3)
                rs = work.tile([128, 2], F32, tag="rs", bufs=6)
                for half in range(max(1, s // 1024)):
                    w = min(1024, s)
                    slab = ps_big.tile([128, 1024], F32, tag="slab")
                    for j in range(w // 512):
                        kc = half * 2 + j
                        nc.tensor.matmul(
                            slab[:, j * 512:(j + 1) * 512],
                            lhsT=QhT[:, h, qt * 128:(qt + 1) * 128],
                            rhs=KhT[:, h, kc * 512:(kc + 1) * 512],
                            start=True, stop=True)
                    nc.scalar.activation(
                        out=exp_slab[:, half * 1024:half * 1024 + w],
                        in_=slab[:, 0:w], func=Exp,
                        accum_out=rs[:, half:half + 1])
                rsum = work.tile([128, 1], F32, tag="rs1", bufs=6)
                if s > 1024:
                    nc.vector.tensor_add(out=rsum, in0=rs[:, 0:1], in1=rs[:, 1:2])
                else:
                    nc.vector.tensor_copy(out=rsum, in_=rs[:, 0:1])
                nc.vector.reciprocal(out=rcol[:, h, qt:qt + 1], in_=rsum)
                nc.vector.tensor_scalar_mul(exp_slab, exp_slab, rcol[:, h, qt:qt + 1])
                nc.sync.dma_start(
                    out=io["attn_out"][h, qt * 128:(qt + 1) * 128, :], in_=exp_slab)

        # transposed orientation + context accumulation, head pair packed.
        # Each sub-head accumulates in its OWN psum bank: a start=True clears
        # the whole bank's has_written bits, so two interleaved accumulation
        # groups must not share a bank.
        for qc in range(C5) if t_on else ():
            ctx_psA = ps_ctx.tile([128, 512], F32, tag="ctx")
            ctx_psB = ps_ctx.tile([128, 512], F32, tag="ctx")
            ctx_out = (ctx_psA[0:64, :], ctx_psB[64:128, :])
            for kt in range(KT):
                slabT = ps_big.tile([128, 1024], F32, tag="slab")
                for sub, h in enumerate(heads):
                    nc.tensor.matmul(
                        slabT[:, sub * 512:(sub + 1) * 512],
                        lhsT=KhT[:, h, kt * 128:(kt + 1) * 128],
                        rhs=QhT[:, h, qc * 512:(qc + 1) * 512],
                        start=True, stop=True)
                expT_t = work.tile([128, 1024], F32, tag="expT", bufs=2)
                nc.scalar.activation(out=expT_t, in_=slabT, func=Exp)
                for sub, h in enumerate(heads):
                    nc.tensor.matmul(
                        ctx_out[sub],
                        lhsT=Vh[:, kt, h, :],
                        rhs=expT_t[:, sub * 512:(sub + 1) * 512],
                        start=(kt == 0), stop=(kt == KT - 1))
            # broadcast recip over the 64 feature partitions and scale ctxT
            rrs = []
            for sub, h in enumerate(heads):
                rr_t = work.tile([1, 512], F32, tag="rr", bufs=2)
                for j in range(4):
                    tr_ps = ps_med.tile([1, 128], F32, tag="med")
                    qt = qc * 4 + j
                    nc.tensor.transpose(
                        out=tr_ps, in_=rcol[:, h, qt:qt + 1], identity=id_sb)
                    nc.scalar.copy(out=rr_t[0:1, j * 128:(j + 1) * 128], in_=tr_ps)
                rrs.append(rr_t)
            bc_ps = ps_med.tile([128, 512], F32, tag="med")
            for sub in range(2):
                nc.tensor.matmul(
                    bc_ps[sub * 64:(sub + 1) * 64, :],
                    lhsT=ones_row[0:1, 0:64], rhs=rrs[sub],
                    start=True, stop=True)
            bc_sb = work.tile([128, 512], F32, tag="bc", bufs=2)
            nc.vector.tensor_copy(out=bc_sb, in_=bc_ps)
            nc.vector.tensor_mul(
                out=ctxT[0:64, pp, qc * 512:(qc + 1) * 512],
                in0=ctx_psA[0:64, :], in1=bc_sb[0:64, :])
            nc.vector.tensor_mul(
                out=ctxT[64:128, pp, qc * 512:(qc + 1) * 512],
                in0=ctx_psB[64:128, :], in1=bc_sb[64:128, :])

    # ---- phase 3: output projection -------------------------------------
    if not (phases & 4):
        return
    wo_ts = []
    for ncol in range(DM // 512):
        wo_t = stage.tile([128, 2, 512], F32, tag="stage")
        src = io["wo"].rearrange(
            "(pp sub r) (ncol j) -> (sub r) pp ncol j", pp=2, sub=2, j=512)
        nc.sync.dma_start(out=wo_t, in_=src[:, :, ncol, :])
        wo_ts.append(wo_t)

    for qt in range(QT):
        out_t = work.tile([128, DM], F32, tag="out_t", bufs=2)
        for ncol in range(DM // 512):
            ps_o = ps_med.tile([128, 512], F32, tag="med")
            # K=128 contraction over a full head pair per matmul; all lhsT at
            # partition base 0 (mixing row-group offsets within one psum
            # accumulation group hangs the HW).
            for pp in range(2):
                nc.tensor.matmul(
                    ps_o,
                    lhsT=ctxT[:, pp, qt * 128:(qt + 1) * 128],
                    rhs=wo_ts[ncol][:, pp, :],
                    start=(pp == 0), stop=(pp == 1))
            nc.vector.tensor_copy(out=out_t[:, ncol * 512:(ncol + 1) * 512], in_=ps_o)
        nc.sync.dma_start(out=io["out_part"][qt * 128:(qt + 1) * 128, :], in_=out_t)


def build_nc(s: int = S, phases: int = 7) -> "bass.Bass":
    nc = bacc.Bacc(
        "TRN2", target_bir_lowering=False, debug=False,
        enable_asserts=False, num_devices=NCORES)
    io = {}
    for name, shape, kind in (
        ("qT", [DIN, s], "ExternalInput"),
        ("kT", [DIN, s], "ExternalInput"),
        ("vT", [DIN, s], "ExternalInput"),
        ("wq", [DIN, HPC * DEP], "ExternalInput"),
        ("wk", [DIN, HPC * DEP], "ExternalInput"),
        ("wv", [DIN, HPC * DEP], "ExternalInput"),
        ("bq", [1, HPC * DEP], "ExternalInput"),
        ("bk", [1, HPC * DEP], "ExternalInput"),
        ("bv", [1, HPC * DEP], "ExternalInput"),
        ("pen", [1, s], "ExternalInput"),
        ("wo", [HPC * DEP, DM], "ExternalInput"),
        ("attn_out", [HPC, s, s], "ExternalOutput"),
        ("out_part", [s, DM], "ExternalOutput"),
    ):
        io[name] = nc.dram_tensor(name, shape, F32, kind=kind).ap()
    with tile.TileContext(nc) as tc:
        with ExitStack() as ctx:
            _emit(ctx, tc, io, s, phases)
    nc.compile()
    return nc


def make_in_maps(inputs: dict, s: int = S) -> list:
    """Slice + transpose the full problem inputs into 8 per-core input maps."""
    f = np.float32
    q, k, v = inputs["q"], inputs["k"], inputs["v"]
    mask = inputs["mask"]
    in_maps = []
    for core in range(NCORES):
        b, hg = core // 4, (core % 4) * HPC
        cols = slice(hg * DEP, (hg + HPC) * DEP)
        m = {
            "qT": np.ascontiguousarray(np.asarray(q[b], f).T),
            "kT": np.ascontiguousarray(np.asarray(k[b], f).T),
            "vT": np.ascontiguousarray(np.asarray(v[b], f).T),
            "wq": np.ascontiguousarray(np.asarray(inputs["Wq"], f)[:, cols] * f(0.125)),
            "wk": np.ascontiguousarray(np.asarray(inputs["Wk"], f)[:, cols]),
            "wv": np.ascontiguousarray(np.asarray(inputs["Wv"], f)[:, cols]),
            "bq": (np.asarray(inputs["bq"], f)[cols] * f(0.125)).reshape(1, -1),
            "bk": np.asarray(inputs["bk"], f)[cols].reshape(1, -1),
            "bv": np.asarray(inputs["bv"], f)[cols].reshape(1, -1),
            "pen": (np.asarray(mask[b, 0, 0], f) * f(PEN)).reshape(1, -1),
            "wo": np.ascontiguousarray(np.asarray(inputs["Wo"], f)[cols, :]),
        }
        in_maps.append({kk: np.ascontiguousarray(vv, dtype=f) for kk, vv in m.items()})
    return in_maps


_NC_CACHE = {}
LAST_RESULTS = None


def _get_nc(s: int = S):
    if s not in _NC_CACHE:
        _NC_CACHE[s] = build_nc(s)
    return _NC_CACHE[s]


def kernel(**inputs) -> tuple:
    nc = _get_nc(S)
    in_maps = make_in_maps(inputs, S)
    res = bass_utils.run_bass_kernel_spmd(nc, in_maps, core_ids=list(range(NCORES)))
    global LAST_RESULTS
    LAST_RESULTS = res

    attn = np.empty((B, H, S, S), np.float32)
    out = np.zeros((B, S, DM), np.float32)
    for core in range(NCORES):
        r = res.results[core]
        b, hg = core // 4, (core % 4) * HPC
        attn[b, hg:hg + HPC] = r["attn_out"]
        out[b] += r["out_part"]
    out += np.asarray(inputs["bo"], np.float32)[None, None, :]
    return out, attn
